# revision 21
# baseline (speedup 1.0000x reference)
"""Trainium2 Bass kernel for nn_Attention_62010737820049 (v3).

Transformer-XL-style relative-position attention block + LN + FFN,
data-parallel over batch across 8 NeuronCores (4 batches per core, no
collectives). All matmuls bf16 at N=512. PSUM consumers (bias adds,
copies, LN stats) run as fused DVE ops; transposes run in bf16; the
rel-pos circulant gather is a skewed DMA re-read of a bf16 DRAM
scratch. The FFN (f1/f2) of batch i is software-pipelined against the
attention front of batch i+1: emission interleaves the two instruction
streams so PE bubbles in one stream (LN chains, softmax, DMA latency)
are filled by matmuls from the other. PSUM is split 4+4 between the
front stream and the f2 accumulators.
"""

import os
import sys

sys.path.insert(0, "/opt/trn_rl_repo")

import numpy as np
import ml_dtypes

B, C, MEM, D = 32, 512, 512, 1024
W = C + MEM           # 1024
FF = 4 * D            # 4096
P = 128
NCORES = 8
BPC = int(os.environ.get("KERNEL_BPC", str(B // NCORES)))  # batches per core
CH_D, CH_C, CH_W, CH_F = D // P, C // P, W // P, FF // P   # 8, 4, 8, 32
EPS = 1e-5
ISQ = 1.0 / 32.0      # 1/sqrt(D)

_cached = {}


def _emit(nc, tc, tn):
    import concourse.bass as bass
    import concourse.mybir as mybir
    from concourse.masks import make_identity

    f32 = mybir.dt.float32
    bf16 = mybir.dt.bfloat16
    AF = mybir.ActivationFunctionType
    OP = mybir.AluOpType

    def vtt(out, a, b, op):
        return nc.vector.tensor_tensor(out=out, in0=a, in1=b, op=op)

    xs, hs, outs = tn["x"], tn["h"], tn["out"]

    with (
        tc.tile_pool(name="constp", bufs=1) as constp,
        tc.tile_pool(name="p2", bufs=28) as p2,      # 2KB/part generic
        tc.tile_pool(name="p1", bufs=20) as p1,      # 1KB/part generic
        tc.tile_pool(name="ftp", bufs=33) as ftp,    # fT tiles
        tc.tile_pool(name="natp", bufs=2) as natp,   # [P,2,D] f32 input stage
        tc.tile_pool(name="wbig", bufs=3) as wbigp,  # [P,4,D] bf16 weight halves
        tc.tile_pool(name="wstr", bufs=3) as wstrp,  # ffn weight stream chunks
        tc.tile_pool(name="p4", bufs=3) as p4,       # [P,D] f32 scratch
        tc.tile_pool(name="stp", bufs=28) as stp,    # [P,1] f32 scalars
        tc.tile_pool(name="psp", bufs=4, space="PSUM") as psp,
        tc.tile_pool(name="psj", bufs=4, space="PSUM") as psjp,
        tc.tile_pool(name="pdram", bufs=2, space="DRAM") as pdram,
    ):
        identb = constp.tile([P, P], bf16, name="identb")
        make_identity(nc, identb[:])
        bq_col = constp.tile([P, CH_D], f32, name="bq_col")
        nc.sync.dma_start(out=bq_col[:], in_=tn["bq"].rearrange("(k p) -> p k", p=P))
        bke_col = constp.tile([P, CH_D], f32, name="bke_col")
        nc.sync.dma_start(out=bke_col[:], in_=tn["bke"].rearrange("(k p) -> p k", p=P))
        bf1_col = constp.tile([P, CH_F], f32, name="bf1_col")
        nc.sync.dma_start(out=bf1_col[:], in_=tn["bf1p"].rearrange("(k p) -> p k", p=P))
        u_sb = constp.tile([P, CH_D, C], bf16, name="u_sb")
        nc.sync.dma_start(out=u_sb[:], in_=tn["u_t"].rearrange("(k p) c -> p k c", p=P))
        ms_sb = constp.tile([P, CH_C, W], bf16, name="ms_sb")
        nc.sync.dma_start(out=ms_sb[:], in_=tn["maskscale"].rearrange("(k p) w -> p k w", p=P))
        vk_sb = constp.tile([P, CH_C, W], bf16, name="vk_sb")
        nc.sync.dma_start(out=vk_sb[:], in_=tn["vkr_ms"].rearrange("(k p) w -> p k w", p=P))
        bv_bc = constp.tile([P, D], bf16, name="bv_bc")
        nc.sync.dma_start(out=bv_bc[:], in_=tn["bv_bc"][:, :])
        bm_bc = constp.tile([P, D], bf16, name="bm_bc")
        nc.sync.dma_start(out=bm_bc[:], in_=tn["bmlp_bc"][:, :])
        bf2_bc = constp.tile([P, D], bf16, name="bf2_bc")
        nc.sync.dma_start(out=bf2_bc[:], in_=tn["bf2_bc"][:, :])

        def load_w(nm):
            h0 = wbigp.tile([P, 4, D], bf16, name=f"{nm}0", tag="w")
            nc.sync.dma_start(
                out=h0[:], in_=tn[nm][0:512, :].rearrange("(k p) d -> p k d", p=P))
            h1 = wbigp.tile([P, 4, D], bf16, name=f"{nm}1", tag="w")
            nc.sync.dma_start(
                out=h1[:], in_=tn[nm][512:1024, :].rearrange("(k p) d -> p k d", p=P))
            return lambda ki: (h0 if ki < 4 else h1)[:, ki % 4, :]

        state = {}

        def front_gen(bi):
            """Phases A..H for batch bi; yields at chunk boundaries."""
            st = state[bi] = {}
            # ---- A: load x,h; cast bf16; transpose into hf_T [d, w] ----
            hf = [p2.tile([P, W], bf16, name=f"hf{dc}", tag="p2") for dc in range(CH_D)]
            for src, woff in ((hs, 0), (xs, C)):
                natb = []
                for cj in range(2):
                    natf = natp.tile([P, 2, D], f32, name="natf", tag="nat")
                    nc.sync.dma_start(
                        out=natf[:],
                        in_=src[bi, cj * 256:(cj + 1) * 256, :].rearrange(
                            "(k p) d -> p k d", p=P))
                    for i in range(2):
                        nb = p2.tile([P, D], bf16, name="natb", tag="p2")
                        nc.vector.tensor_copy(out=nb[:], in_=natf[:, i, :])
                        natb.append(nb)
                yield
                for dc in range(CH_D):
                    tp = psp.tile([P, C], bf16, name="tpA", tag="ps")
                    for ci in range(CH_C):
                        nc.tensor.transpose(
                            tp[:, ci * P:(ci + 1) * P],
                            natb[ci][:, dc * P:(dc + 1) * P],
                            identb[:])
                    nc.vector.tensor_copy(out=hf[dc][:, woff:woff + C], in_=tp[:])
                    if dc % 4 == 3:
                        yield
            # ---- B: quT = Wq x_T + bq + u_T ----
            wq = load_w("wq_t")
            quT = []
            for do in range(CH_D):
                qps = psp.tile([P, C], f32, name="qps", tag="ps")
                for ki in range(CH_D):
                    nc.tensor.matmul(
                        qps[:], wq(ki)[:, do * P:(do + 1) * P], hf[ki][:, C:W],
                        start=(ki == 0), stop=(ki == CH_D - 1))
                qu = p1.tile([P, C], bf16, name=f"quT{do}", tag="p1")
                nc.vector.scalar_tensor_tensor(
                    out=qu[:], in0=qps[:], scalar=bq_col[:, do:do + 1],
                    in1=u_sb[:, do, :], op0=OP.add, op1=OP.add)
                quT.append(qu)
                yield
            # ---- C: k_T = Wke hf_T + bke ----
            wke = load_w("wke_t")
            kT = []
            for do in range(CH_D):
                kt = p2.tile([P, W], bf16, name=f"kT{do}", tag="p2")
                for hh in range(2):
                    kps = psp.tile([P, C], f32, name="kps", tag="ps")
                    for ki in range(CH_D):
                        nc.tensor.matmul(
                            kps[:], wke(ki)[:, do * P:(do + 1) * P],
                            hf[ki][:, hh * 512:(hh + 1) * 512],
                            start=(ki == 0), stop=(ki == CH_D - 1))
                    nc.vector.tensor_scalar_add(
                        kt[:, hh * 512:(hh + 1) * 512], kps[:],
                        bke_col[:, do:do + 1])
                kT.append(kt)
                yield
            # ---- E: P = qu @ kr^T -> DRAM scratch (bf16) ----
            kr = load_w("kr_t")
            pd = pdram.tile([C * W], bf16, name="pd", tag="pd")
            pd2 = pd.rearrange("(c w) -> c w", w=W)
            for ci in range(CH_C):
                pcb = p2.tile([P, W], bf16, name="pcb", tag="p2")
                for hh in range(2):
                    pps = psp.tile([P, C], f32, name="pps", tag="ps")
                    for ki in range(CH_D):
                        nc.tensor.matmul(
                            pps[:], quT[ki][:, ci * P:(ci + 1) * P],
                            kr(ki)[:, hh * 512:(hh + 1) * 512],
                            start=(ki == 0), stop=(ki == CH_D - 1))
                    nc.vector.tensor_copy(
                        out=pcb[:, hh * 512:(hh + 1) * 512], in_=pps[:])
                nc.sync.dma_start(out=pd2[ci * P:(ci + 1) * P, :], in_=pcb[:])
                yield
            # ---- F: att = (qu@k^T + skew(P))*ms + vk2, softmax, transpose ----
            attn = []
            for ci in range(CH_C):
                ah = []
                for hh in range(2):
                    aps = psp.tile([P, C], f32, name="aps", tag="ps")
                    for ki in range(CH_D):
                        nc.tensor.matmul(
                            aps[:], quT[ki][:, ci * P:(ci + 1) * P],
                            kT[ki][:, hh * 512:(hh + 1) * 512],
                            start=(ki == 0), stop=(ki == CH_D - 1))
                    ah.append(aps)
                pskew = p2.tile([P, W], bf16, name="pskew", tag="p2")
                skew_ap = bass.AP(
                    tensor=pd.tensor,
                    offset=pd.offset + (W - 1) * P * ci + C - 1,
                    ap=[[W - 1, P], [1, W]])
                nc.sync.dma_start(out=pskew[:], in_=skew_ap)
                t = p4.tile([P, W], f32, name="t", tag="p4")
                for hh in range(2):
                    vtt(t[:, hh * 512:(hh + 1) * 512], ah[hh][:],
                        pskew[:, hh * 512:(hh + 1) * 512], OP.add)
                vtt(t[:], t[:], ms_sb[:, ci, :], OP.mult)
                vtt(t[:], t[:], vk_sb[:, ci, :], OP.add)
                e = p2.tile([P, W], bf16, name=f"attn{ci}", tag="p2")
                zrow = stp.tile([P, 1], f32, name="zrow", tag="st")
                nc.scalar.activation(e[:], t[:], AF.Exp, accum_out=zrow[:])
                rz = stp.tile([P, 1], f32, name="rz", tag="st")
                nc.vector.reciprocal(rz[:], zrow[:])
                nc.vector.tensor_scalar_mul(e[:], e[:], rz[:])
                attn.append(e)
                yield
            attT = []
            for wc in range(CH_W):
                tp = psp.tile([P, C], bf16, name="tpF", tag="ps")
                for ci in range(CH_C):
                    nc.tensor.transpose(
                        tp[:, ci * P:(ci + 1) * P],
                        attn[ci][:, wc * P:(wc + 1) * P],
                        identb[:])
                at = p1.tile([P, C], bf16, name=f"attT{wc}", tag="p1")
                nc.vector.tensor_copy(out=at[:], in_=tp[:])
                attT.append(at)
                if wc % 2 == 1:
                    yield
            # ---- D: val = hf @ Wv^T + bv  (natural [w, d]) ----
            wv = load_w("wv_t")
            val = []
            for wc in range(CH_W):
                vt = p2.tile([P, D], bf16, name=f"val{wc}", tag="p2")
                for hh in range(2):
                    vps = psp.tile([P, C], f32, name="vps", tag="ps")
                    for ki in range(CH_D):
                        nc.tensor.matmul(
                            vps[:], hf[ki][:, wc * P:(wc + 1) * P],
                            wv(ki)[:, hh * 512:(hh + 1) * 512],
                            start=(ki == 0), stop=(ki == CH_D - 1))
                    vtt(vt[:, hh * 512:(hh + 1) * 512], vps[:],
                        bv_bc[:, hh * 512:(hh + 1) * 512], OP.add)
                val.append(vt)
                yield
            # ---- G: o_T = val^T @ att^T  [d, c] ----
            oT = []
            for do in range(CH_D):
                ops = psp.tile([P, C], f32, name="ops", tag="ps")
                for wc in range(CH_W):
                    nc.tensor.matmul(
                        ops[:], val[wc][:, do * P:(do + 1) * P], attT[wc][:],
                        start=(wc == 0), stop=(wc == CH_W - 1))
                ot = p1.tile([P, C], bf16, name=f"oT{do}", tag="p1")
                nc.vector.tensor_copy(out=ot[:], in_=ops[:])
                oT.append(ot)
                yield
            # ---- H: o2 = o @ Wmlp^T + bmlp ; LN -> z_T ----
            wm = load_w("wmlp_t")
            o2s, svals = [], []
            for ci in range(CH_C):
                o2 = p2.tile([P, D], bf16, name=f"o2_{ci}", tag="p2")
                s = []
                for hh in range(2):
                    o2ps = psp.tile([P, C], f32, name="o2ps", tag="ps")
                    for ki in range(CH_D):
                        nc.tensor.matmul(
                            o2ps[:], oT[ki][:, ci * P:(ci + 1) * P],
                            wm(ki)[:, hh * 512:(hh + 1) * 512],
                            start=(ki == 0), stop=(ki == CH_D - 1))
                    sh = stp.tile([P, 1], f32, name="sh", tag="st")
                    nc.vector.scalar_tensor_tensor(
                        out=o2[:, hh * 512:(hh + 1) * 512], in0=o2ps[:],
                        scalar=1.0, in1=bm_bc[:, hh * 512:(hh + 1) * 512],
                        op0=OP.mult, op1=OP.add, accum_out=sh[:])
                    s.append(sh)
                o2s.append(o2)
                svals.append(s)
                yield
            zs = []
            for ci in range(CH_C):
                o2, s = o2s[ci], svals[ci]
                sq = p2.tile([P, W], bf16, name="sqscr", tag="p2")
                ss0 = stp.tile([P, 1], f32, name="ss0", tag="st")
                ss1 = stp.tile([P, 1], f32, name="ss1", tag="st")
                nc.scalar.activation(sq[:, 0:512], o2[:, 0:512], AF.Square,
                                     bias=0.0, accum_out=ss0[:])
                nc.scalar.activation(sq[:, 512:1024], o2[:, 512:1024], AF.Square,
                                     bias=0.0, accum_out=ss1[:])
                mu = stp.tile([P, 1], f32, name="mu", tag="st")
                vtt(mu[:], s[0][:], s[1][:], OP.add)
                nc.vector.tensor_scalar_mul(mu[:], mu[:], 1.0 / D)
                ex2 = stp.tile([P, 1], f32, name="ex2", tag="st")
                vtt(ex2[:], ss0[:], ss1[:], OP.add)
                nc.vector.tensor_scalar_mul(ex2[:], ex2[:], 1.0 / D)
                var = stp.tile([P, 1], f32, name="var", tag="st")
                vtt(var[:], mu[:], mu[:], OP.mult)
                vtt(var[:], ex2[:], var[:], OP.subtract)
                nc.vector.tensor_scalar_add(var[:], var[:], EPS)
                sd = stp.tile([P, 1], f32, name="sd", tag="st")
                nc.scalar.activation(sd[:], var[:], AF.Sqrt, bias=0.0)
                rstd = stp.tile([P, 1], f32, name="rstd", tag="st")
                nc.vector.reciprocal(rstd[:], sd[:])
                zc = p2.tile([P, D], bf16, name=f"zc{ci}", tag="p2")
                nc.vector.tensor_scalar(
                    out=zc[:], in0=o2[:], scalar1=mu[:], scalar2=rstd[:],
                    op0=OP.subtract, op1=OP.mult)
                zs.append(zc)
                yield
            zT = []
            for dc in range(CH_D):
                tp = psp.tile([P, C], bf16, name="tpH", tag="ps")
                for ci in range(CH_C):
                    nc.tensor.transpose(
                        tp[:, ci * P:(ci + 1) * P],
                        zs[ci][:, dc * P:(dc + 1) * P],
                        identb[:])
                zt = p1.tile([P, C], bf16, name=f"zT{dc}", tag="p1")
                nc.vector.tensor_copy(out=zt[:], in_=tp[:])
                zT.append(zt)
                if dc % 2 == 1:
                    yield
            st["zT"] = zT

        def ffn_gen(bi):
            """Phases I (f1+relu) and J (f2+out) for batch bi; yields at chunks."""
            zT = state[bi]["zT"]
            # ---- I: f_T = relu(Wf1g z_T + bf1') ----
            fT = []
            for jc2 in range(CH_F // 2):
                w1c = wstrp.tile([P, CH_D, 2 * P], bf16, name="w1c", tag="ws")
                nc.scalar.dma_start(
                    out=w1c[:],
                    in_=tn["wf1_t"][:, jc2 * 256:(jc2 + 1) * 256].rearrange(
                        "(dc p) j -> p dc j", p=P))
                for j2 in range(2):
                    jc = jc2 * 2 + j2
                    fps = psp.tile([P, C], f32, name="fps", tag="ps")
                    for dc in range(CH_D):
                        nc.tensor.matmul(
                            fps[:], w1c[:, dc, j2 * P:(j2 + 1) * P], zT[dc][:],
                            start=(dc == 0), stop=(dc == CH_D - 1))
                    ft = ftp.tile([P, C], bf16, name=f"fT{jc}", tag="ft")
                    nc.scalar.activation(ft[:], fps[:], AF.Relu,
                                         bias=bf1_col[:, jc:jc + 1])
                    fT.append(ft)
                yield
            # ---- J: out = f @ Wf2^T + bf2, two ci-pair half-passes ----
            for half in range(2):
                cis = (0, 1) if half == 0 else (2, 3)
                outps = [psjp.tile([P, C], f32, name=f"outps{half}{i}", tag="pj")
                         for i in range(4)]
                for jc2 in range(CH_F // 2):
                    w2c = wstrp.tile([P, 2, D], bf16, name="w2c", tag="ws")
                    nc.scalar.dma_start(
                        out=w2c[:],
                        in_=tn["wf2_b"][jc2 * 256:(jc2 + 1) * 256, :].rearrange(
                            "(k p) d -> p k d", p=P))
                    for j2 in range(2):
                        jc = jc2 * 2 + j2
                        for hh in range(2):
                            for li, ci in enumerate(cis):
                                nc.tensor.matmul(
                                    outps[li * 2 + hh][:],
                                    fT[jc][:, ci * P:(ci + 1) * P],
                                    w2c[:, j2, hh * 512:(hh + 1) * 512],
                                    start=(jc == 0), stop=(jc == CH_F - 1))
                        yield
                for li, ci in enumerate(cis):
                    ob = p4.tile([P, D], f32, name="ob", tag="p4")
                    for hh in range(2):
                        vtt(ob[:, hh * 512:(hh + 1) * 512],
                            outps[li * 2 + hh][:],
                            bf2_bc[:, hh * 512:(hh + 1) * 512], OP.add)
                    nc.sync.dma_start(out=outs[bi, ci * P:(ci + 1) * P, :], in_=ob[:])
                yield

        def _one_pass():
            state.clear()
            fronts = {0: front_gen(0)}
            for _ in fronts[0]:
                pass
            for bi in range(BPC):
                g_ffn = ffn_gen(bi)
                g_front = front_gen(bi + 1) if bi + 1 < BPC else None
                done_ffn = done_front = False
                while not (done_ffn and done_front):
                    if not done_ffn:
                        if next(g_ffn, "END") == "END":
                            done_ffn = True
                    if g_front is not None and not done_front:
                        for _ in range(2):
                            if next(g_front, "END") == "END":
                                done_front = True
                                break
                    else:
                        done_front = True

        LOOP_R = int(os.environ.get("KERNEL_LOOP", "0"))
        UNROLL = max(1, int(os.environ.get("KERNEL_UNROLL", "0")))
        if LOOP_R > 1:
            with tc.For_i(0, LOOP_R, 1):
                for _ in range(UNROLL):
                    _one_pass()
        else:
            for _ in range(UNROLL):
                _one_pass()


def _build():
    if "nc" in _cached:
        return _cached["nc"]
    import concourse.mybir as mybir
    import concourse.tile as tile
    from concourse import bacc

    f32 = mybir.dt.float32
    bf16 = mybir.dt.bfloat16
    nc = bacc.Bacc("TRN2", target_bir_lowering=False, debug=False,
                   num_devices=NCORES)
    tn = {}
    tn["x"] = nc.dram_tensor("x", [BPC, C, D], f32, kind="ExternalInput")
    tn["h"] = nc.dram_tensor("h", [BPC, MEM, D], f32, kind="ExternalInput")
    for nm, shp in [
        ("wq_t", [D, D]), ("wke_t", [D, D]), ("wv_t", [D, D]), ("wmlp_t", [D, D]),
        ("kr_t", [D, W]), ("u_t", [D, C]), ("wf1_t", [D, FF]), ("wf2_b", [FF, D]),
        ("maskscale", [C, W]), ("vkr_ms", [C, W]),
        ("bv_bc", [P, D]), ("bmlp_bc", [P, D]), ("bf2_bc", [P, D]),
    ]:
        tn[nm] = nc.dram_tensor(nm, shp, bf16, kind="ExternalInput")
    for nm, shp in [
        ("bq", [D]), ("bke", [D]), ("bf1p", [FF]),
    ]:
        tn[nm] = nc.dram_tensor(nm, shp, f32, kind="ExternalInput")
    tn["out"] = nc.dram_tensor("out", [BPC, C, D], f32, kind="ExternalOutput")

    with tile.TileContext(nc) as tc:
        _emit(nc, tc, tn)
    nc.compile()
    _cached["nc"] = nc
    return nc


def _host_consts(inputs):
    f = np.float32
    b16 = ml_dtypes.bfloat16
    Wq, bq = inputs["Wq"].astype(f), inputs["bq"].astype(f)
    Wke, bke = inputs["Wke"].astype(f), inputs["bke"].astype(f)
    Wkr, bkr = inputs["Wkr"].astype(f), inputs["bkr"].astype(f)
    Wv, bv = inputs["Wv"].astype(f), inputs["bv"].astype(f)
    Wmlp, bmlp = inputs["Wmlp"].astype(f), inputs["bmlp"].astype(f)
    gamma, beta = inputs["gamma"].astype(f), inputs["beta"].astype(f)
    Wf1, bf1 = inputs["Wf1"].astype(f), inputs["bf1"].astype(f)
    Wf2, bf2 = inputs["Wf2"].astype(f), inputs["bf2"].astype(f)
    u, v, rr = inputs["u"].astype(f), inputs["v"].astype(f), inputs["r"].astype(f)

    kr = rr @ Wkr.T + bkr                      # [W, D]
    vkr = v @ kr.T                             # [C, W]
    # circulant skew of u @ kr^T folded into the additive constant
    i = np.roll(np.arange(W), -C + 1)[::-1]
    i2 = np.concatenate([i, i])
    win = np.lib.stride_tricks.sliding_window_view(i2, W)[:, ::-1]
    idx = np.ascontiguousarray(win[:C])        # [C, W]
    ukr = u @ kr.T                             # [C, W]
    ukr_s = np.take_along_axis(ukr, idx, axis=1)
    mask = (np.arange(W)[None, :] <= np.arange(C)[:, None] + MEM)
    maskscale = (mask * ISQ).astype(f)
    vk2 = ((vkr - ukr_s) * maskscale).astype(f)
    cn = {
        "wq_t": np.ascontiguousarray(Wq.T).astype(b16),
        "wke_t": np.ascontiguousarray(Wke.T).astype(b16),
        "wv_t": np.ascontiguousarray(Wv.T).astype(b16),
        "wmlp_t": np.ascontiguousarray(Wmlp.T).astype(b16),
        "kr_t": np.ascontiguousarray(kr.T).astype(b16),
        "u_t": np.ascontiguousarray(u.T).astype(b16),
        "wf1_t": np.ascontiguousarray((Wf1 * gamma[None, :]).T).astype(b16),
        "wf2_b": np.ascontiguousarray(Wf2.T).astype(b16),
        "maskscale": maskscale.astype(b16),
        "vkr_ms": vk2.astype(b16),
        "bq": bq, "bke": bke,
        "bf1p": (bf1 + Wf1 @ beta).astype(f),
        "bv_bc": np.tile(bv.reshape(1, D), (P, 1)).astype(b16),
        "bmlp_bc": np.tile(bmlp.reshape(1, D), (P, 1)).astype(b16),
        "bf2_bc": np.tile(bf2.reshape(1, D), (P, 1)).astype(b16),
    }
    return cn


def kernel(**inputs):
    from concourse.bass_utils import run_bass_kernel_spmd

    nc = _build()
    cn = _host_consts(inputs)
    x = np.ascontiguousarray(inputs["x"].astype(np.float32))
    h = np.ascontiguousarray(inputs["h"].astype(np.float32))
    in_maps = []
    for i in range(NCORES):
        m = dict(cn)
        m["x"] = np.ascontiguousarray(x[i * BPC:(i + 1) * BPC])
        m["h"] = np.ascontiguousarray(h[i * BPC:(i + 1) * BPC])
        in_maps.append(m)
    res = run_bass_kernel_spmd(nc, in_maps, list(range(NCORES)))
    out = np.concatenate([res.results[i]["out"] for i in range(NCORES)], axis=0)
    return out.astype(np.float32)
